# revision 1
# baseline (speedup 1.0000x reference)
"""DSGIAT GraphBranch kernel for trn2 (8 NeuronCores).

Device: channel-sharded conv1 GEMM (x @ W1 shard + folded attention-logit
columns) across 8 cores via Bass/Tile. Host: edge aggregation via sorted
segment reduceat, LP diffusion, pooling, MLP.
"""
import numpy as np
from contextlib import ExitStack

N_NODES = 30000
N_PAD = 30080          # 235 * 128
N_TILES = 235
IN_CH = 256
HID = 128
HEADS = 4
OUT1 = 512
N_GRAPHS = 64
LP_ALPHA = 0.5
NEG_SLOPE = 0.2
EPS = 1e-16
N_CORES = 8
SHARD = OUT1 // N_CORES  # 64

_cached = {}


def _build_device_program():
    import concourse.tile as tile
    from concourse import bacc, mybir

    nc = bacc.Bacc("TRN2", target_bir_lowering=False, debug=False,
                   num_devices=N_CORES)
    xT = nc.dram_tensor("xT", [IN_CH, N_PAD], mybir.dt.float32,
                        kind="ExternalInput")
    w1rhs = nc.dram_tensor("w1rhs", [IN_CH, 66], mybir.dt.float32,
                           kind="ExternalInput")
    out_h = nc.dram_tensor("out_h", [N_PAD, 66], mybir.dt.float32,
                           kind="ExternalOutput")

    with tile.TileContext(nc) as tc, ExitStack() as ctx:
        sb = ctx.enter_context(tc.tile_pool(name="sb", bufs=4))
        wp = ctx.enter_context(tc.tile_pool(name="wp", bufs=1))
        ps = ctx.enter_context(tc.tile_pool(name="ps", bufs=4, space="PSUM"))

        w_sb = wp.tile([128, 2, 66], mybir.dt.float32)
        nc.sync.dma_start(w_sb[:, 0, :], w1rhs[0:128, :])
        nc.sync.dma_start(w_sb[:, 1, :], w1rhs[128:256, :])

        for j in range(N_TILES):
            acc = ps.tile([128, 66], mybir.dt.float32, space="PSUM")
            for q in range(2):
                lhsT = sb.tile([128, 128], mybir.dt.float32, tag="lhsT")
                nc.sync.dma_start(
                    lhsT[:], xT[q * 128:(q + 1) * 128, j * 128:(j + 1) * 128])
                nc.tensor.matmul(acc[:], lhsT=lhsT[:], rhs=w_sb[:, q, :],
                                 start=(q == 0), stop=(q == 1))
            res = sb.tile([128, 66], mybir.dt.float32, tag="res")
            nc.vector.tensor_copy(res[:], acc[:])
            nc.sync.dma_start(out_h[j * 128:(j + 1) * 128, :], res[:])
    nc.compile()
    return nc


def _run_device(xT, w1rhs_list):
    from concourse.bass_utils import run_bass_kernel_spmd
    if "nc" not in _cached:
        _cached["nc"] = _build_device_program()
    nc = _cached["nc"]
    in_maps = [{"xT": xT, "w1rhs": w1rhs_list[c]} for c in range(N_CORES)]
    import time
    t0 = time.time()
    res = run_bass_kernel_spmd(nc, in_maps, core_ids=list(range(N_CORES)))
    _cached["device_wall_ns"] = int((time.time() - t0) * 1e9)
    _cached["last_result"] = res
    return [res.results[c]["out_h"] for c in range(N_CORES)]


def _seg_sum(vals, starts, n_seg):
    """Segment sum of vals over sorted segments; starts has n_seg entries."""
    out = np.add.reduceat(vals, starts, axis=0)
    # fix empty segments (reduceat returns vals[start] there)
    counts = np.diff(np.append(starts, len(vals)))
    if vals.ndim == 1:
        out = np.where(counts > 0, out, 0.0)
    else:
        out = np.where((counts > 0)[:, None], out, 0.0)
    return out


def _seg_max(vals, starts, n_seg):
    out = np.maximum.reduceat(vals, starts, axis=0)
    counts = np.diff(np.append(starts, len(vals)))
    out = np.where((counts > 0)[:, None], out, 0.0)
    return out


def _sorted_edges(src, dst, n):
    order = np.argsort(dst, kind="stable")
    s, d = src[order], dst[order]
    starts = np.searchsorted(d, np.arange(n))
    return s, d, starts


def _gat_agg(h, es, ed, src, dst, n):
    """h [N,512], es/ed [N,4]; edges include self loops, any order."""
    s, d, starts = _sorted_edges(src, dst, n)
    e = es[s] + ed[d]                                 # [E,4]
    e = np.where(e >= 0, e, NEG_SLOPE * e)
    m = _seg_max(e, starts, n)                        # [N,4]
    a = np.exp(e - m[d])                              # [E,4]
    denom = _seg_sum(a, starts, n)                    # [N,4]
    hh = h.reshape(n, HEADS, HID)
    msg = hh[s] * a[:, :, None]                       # [E,4,128]
    outs = _seg_sum(msg.reshape(len(s), -1), starts, n).reshape(n, HEADS, HID)
    outs = outs / (denom[:, :, None] + EPS)
    return outs.reshape(n, OUT1)


def _label_prop(y, src, dst, dis, n):
    s, d, starts = _sorted_edges(src, dst, n)
    w = (dis[s] * dis[d])[:, None]
    res = (1.0 - LP_ALPHA) * y
    out = y
    for _ in range(2):
        agg = _seg_sum(out[s] * w, starts, n)
        out = np.clip(LP_ALPHA * agg + res, 0.0, 1.0)
    return out


def kernel(x, edge_index, batch,
           conv1_W, conv1_asrc, conv1_adst, conv1_b,
           conv2_W, conv2_asrc, conv2_adst, conv2_b,
           mlp_W1, mlp_b1, mlp_W2, mlp_b2):
    x = np.asarray(x, dtype=np.float32)
    edge_index = np.asarray(edge_index)
    batch = np.asarray(batch)
    conv1_W = np.asarray(conv1_W, np.float32)
    conv2_W = np.asarray(conv2_W, np.float32)
    n = x.shape[0]
    src = edge_index[0].astype(np.int64)
    dst = edge_index[1].astype(np.int64)
    loop = np.arange(n, dtype=np.int64)
    c_src = np.concatenate([src, loop])
    c_dst = np.concatenate([dst, loop])

    # degrees / LP normalization
    deg = np.bincount(dst, minlength=n).astype(np.float32)
    dis = np.where(deg > 0, 1.0 / np.sqrt(np.maximum(deg, 1.0)), 0.0)

    # ---- device phase: conv1 GEMM, channel-sharded across 8 cores ----
    # folded logit weights: e_s = x @ (W1_head @ a_src)
    a1s = np.asarray(conv1_asrc, np.float32)  # [4,128]
    a1d = np.asarray(conv1_adst, np.float32)
    w_es1 = np.stack([conv1_W[:, h * HID:(h + 1) * HID] @ a1s[h]
                      for h in range(HEADS)], axis=1)  # [256,4]
    w_ed1 = np.stack([conv1_W[:, h * HID:(h + 1) * HID] @ a1d[h]
                      for h in range(HEADS)], axis=1)

    xp = np.zeros((N_PAD, IN_CH), dtype=np.float32)
    xp[:n] = x
    xT = np.ascontiguousarray(xp.T)  # [256, 30080]
    w1rhs_list = []
    for c in range(N_CORES):
        w = np.zeros((IN_CH, 66), dtype=np.float32)
        w[:, 0:SHARD] = conv1_W[:, c * SHARD:(c + 1) * SHARD]
        h_head = c // 2
        w[:, 64] = w_es1[:, h_head]
        w[:, 65] = w_ed1[:, h_head]
        w1rhs_list.append(w)

    outs = _run_device(xT, w1rhs_list)
    h1_pre = np.concatenate([o[:n, 0:SHARD] for o in outs], axis=1)  # [N,512]
    es1 = np.stack([outs[2 * h][:n, 64] for h in range(HEADS)], axis=1)
    ed1 = np.stack([outs[2 * h][:n, 65] for h in range(HEADS)], axis=1)

    # ---- host: conv1 aggregation + relu + LP ----
    h1 = _gat_agg(h1_pre, es1, ed1, c_src, c_dst, n) + np.asarray(conv1_b, np.float32)
    h1 = np.maximum(h1, 0.0)
    h1 = _label_prop(h1, src, dst, dis, n)

    # ---- host: conv2 ----
    h2_pre = h1 @ conv2_W
    a2s = np.asarray(conv2_asrc, np.float32)
    a2d = np.asarray(conv2_adst, np.float32)
    hh = h2_pre.reshape(n, HEADS, HID)
    es2 = np.einsum("nhc,hc->nh", hh, a2s)
    ed2 = np.einsum("nhc,hc->nh", hh, a2d)
    h2 = _gat_agg(h2_pre, es2, ed2, c_src, c_dst, n) + np.asarray(conv2_b, np.float32)
    h2 = np.maximum(h2, 0.0)
    h2 = _label_prop(h2, src, dst, dis, n)

    # ---- pooling + MLP ----
    combined = np.concatenate([x, h1, h2], axis=1)  # [N,1280]
    b = batch.astype(np.int64)
    sums = np.zeros((N_GRAPHS, combined.shape[1]), dtype=np.float32)
    np.add.at(sums, b, combined)
    cnts = np.bincount(b, minlength=N_GRAPHS).astype(np.float32)
    pooled = sums / np.maximum(cnts, 1.0)[:, None]
    hdd = np.maximum(pooled @ np.asarray(mlp_W1, np.float32)
                     + np.asarray(mlp_b1, np.float32), 0.0)
    out = hdd @ np.asarray(mlp_W2, np.float32) + np.asarray(mlp_b2, np.float32)
    return out.astype(np.float32)



# revision 5
# speedup vs baseline: 1.0983x; 1.0983x over previous
"""DSGIAT GraphBranch kernel for trn2, 8 NeuronCores, full model on device.

Design: node-sharded (3840 padded nodes/core). Edge aggregation is done per
128-dst-node panel: gather source rows via indirect DMA, build a 0/1
selection matrix from dst indices (is_equal vs iota), and segment-sum via
TensorE matmul (sel.T @ msg). Stage boundaries that need the full node table
(conv outputs feeding LP gathers, GEMM tables feeding conv gathers) are
replicated via AllGather. Host does edge sorting/packing and the tiny MLP.
"""
import numpy as np
import ml_dtypes
from contextlib import ExitStack

N_NODES = 30000
N_PAD = 30720            # 8 * 3840
N_CORES = 8
NC_PAD = N_PAD // N_CORES     # 3840 rows per core
PPC = NC_PAD // 128           # 30 panels per core
IN_CH = 256
HID = 128
HEADS = 4
OUT1 = 512
TW = OUT1 + 2 * HEADS         # 520 table width: [h | es | ed]
N_GRAPHS = 64
LP_ALPHA = 0.5
NEG = 0.2

_cached = {}


def _build_program(K1, K2):
    import concourse.tile as tile
    from concourse import bacc, bass, mybir

    BF16 = mybir.dt.bfloat16
    F32 = mybir.dt.float32
    I32 = mybir.dt.int32

    nc = bacc.Bacc("TRN2", target_bir_lowering=False, debug=False,
                   num_devices=N_CORES)

    # ---- inputs (per core) ----
    xTs = nc.dram_tensor("xTs", [2, 128, PPC, 128], BF16, kind="ExternalInput")
    xs = nc.dram_tensor("xs", [PPC, 128, IN_CH], BF16, kind="ExternalInput")
    wcat1 = nc.dram_tensor("wcat1", [2, 128, TW], BF16, kind="ExternalInput")
    wcat2 = nc.dram_tensor("wcat2", [4, 128, TW], BF16, kind="ExternalInput")
    csrc = nc.dram_tensor("csrc", [PPC, 128, K1], I32, kind="ExternalInput")
    cdst = nc.dram_tensor("cdst", [PPC, 128, K1], I32, kind="ExternalInput")
    lsrc = nc.dram_tensor("lsrc", [PPC, 128, K2], I32, kind="ExternalInput")
    ldst = nc.dram_tensor("ldst", [PPC, 128, K2], I32, kind="ExternalInput")
    lw = nc.dram_tensor("lw", [PPC, 128, K2], F32, kind="ExternalInput")
    batchl = nc.dram_tensor("batchl", [PPC, 128, 1], I32, kind="ExternalInput")
    iota = nc.dram_tensor("iota", [128, 128], I32, kind="ExternalInput")
    ident = nc.dram_tensor("ident", [128, 128], BF16, kind="ExternalInput")

    out_pool = nc.dram_tensor("out_pool", [64, 1280], F32, kind="ExternalOutput")

    RG = [list(range(N_CORES))]

    # persistent DRAM scratch (plain Internal tensors: indirect DMA needs
    # zero-offset APs)
    t1in = nc.dram_tensor("t1in", [PPC, 128, TW], BF16, kind="Internal")
    T1 = nc.dram_tensor("T1", [N_PAD, TW], BF16, kind="Internal", addr_space="Shared")
    y0in = nc.dram_tensor("y0in", [PPC, 128, OUT1], BF16, kind="Internal")
    Y0 = nc.dram_tensor("Y0", [N_PAD, OUT1], BF16, kind="Internal", addr_space="Shared")
    y1ain = nc.dram_tensor("y1ain", [PPC, 128, OUT1], BF16, kind="Internal")
    Y1a = nc.dram_tensor("Y1a", [N_PAD, OUT1], BF16, kind="Internal", addr_space="Shared")
    y1bin = nc.dram_tensor("y1bin", [PPC, 128, OUT1], BF16, kind="Internal")
    t2in = nc.dram_tensor("t2in", [PPC, 128, TW], BF16, kind="Internal")
    T2 = nc.dram_tensor("T2", [N_PAD, TW], BF16, kind="Internal", addr_space="Shared")
    y20in = nc.dram_tensor("y20in", [PPC, 128, OUT1], BF16, kind="Internal")
    Y20 = nc.dram_tensor("Y20", [N_PAD, OUT1], BF16, kind="Internal", addr_space="Shared")
    y2ain = nc.dram_tensor("y2ain", [PPC, 128, OUT1], BF16, kind="Internal")
    Y2a = nc.dram_tensor("Y2a", [N_PAD, OUT1], BF16, kind="Internal", addr_space="Shared")
    y2bin = nc.dram_tensor("y2bin", [PPC, 128, OUT1], BF16, kind="Internal")
    res1 = nc.dram_tensor("res1", [PPC, 128, OUT1], BF16, kind="Internal")
    res2 = nc.dram_tensor("res2", [PPC, 128, OUT1], BF16, kind="Internal")

    phase_n = [0]

    with tile.TileContext(nc) as tc, ExitStack() as ctx:
        cpool = ctx.enter_context(tc.tile_pool(name="cpool", bufs=1))

        # constants
        iota_t = cpool.tile([128, 128], I32)
        nc.sync.dma_start(iota_t[:], iota[:])
        ident_t = cpool.tile([128, 128], BF16)
        nc.sync.dma_start(ident_t[:], ident[:])

        def gemm(dst_dram, lhs_loader, w_dram, kt):
            """dst_dram[j] = lhsT_j.T @ wcat  for j in range(PPC)."""
            with ExitStack() as c2:
                pn = phase_n[0]; phase_n[0] += 1
                sb = c2.enter_context(tc.tile_pool(name=f"gsb{pn}", bufs=3))
                wp = c2.enter_context(tc.tile_pool(name=f"gwp{pn}", bufs=1))
                ps = c2.enter_context(tc.tile_pool(name=f"gps{pn}", bufs=1, space="PSUM"))
                w_t = wp.tile([128, kt, TW], BF16)
                for q in range(kt):
                    nc.sync.dma_start(w_t[:, q, :], w_dram[q])
                for j in range(PPC):
                    p1 = ps.tile([128, OUT1], F32, tag="p1", bufs=2)
                    p2 = ps.tile([128, 8], F32, tag="p2", bufs=2)
                    for q in range(kt):
                        lt = sb.tile([128, 128], BF16, tag="lt", bufs=3)
                        lhs_loader(lt, j, q)
                        nc.tensor.matmul(p1[:], lhsT=lt[:], rhs=w_t[:, q, 0:OUT1],
                                         start=(q == 0), stop=(q == kt - 1))
                        nc.tensor.matmul(p2[:], lhsT=lt[:], rhs=w_t[:, q, OUT1:TW],
                                         start=(q == 0), stop=(q == kt - 1))
                    st = sb.tile([128, TW], BF16, tag="st", bufs=3)
                    nc.scalar.copy(st[:, 0:OUT1], p1[:])
                    nc.scalar.copy(st[:, OUT1:TW], p2[:])
                    nc.sync.dma_start(dst_dram[j], st[:])

        def allgather(src3, dst2):
            nc.gpsimd.collective_compute(
                "AllGather", mybir.AluOpType.bypass, replica_groups=RG,
                ins=[src3[:].opt()], outs=[dst2[:].opt()])

        def conv_agg(T, tin, yin, res):
            """GAT aggregation: panels of 128 dst, K1 edge tiles each."""
            with ExitStack() as c2:
                pn = phase_n[0]; phase_n[0] += 1
                sb = c2.enter_context(tc.tile_pool(name=f"casb{pn}", bufs=2))
                ps = c2.enter_context(tc.tile_pool(name=f"caps{pn}", bufs=1, space="PSUM"))
                with tc.For_i(0, PPC, 1) as i:
                    src_t = sb.tile([128, K1], I32, tag="srct", bufs=2)
                    nc.sync.dma_start(src_t[:], csrc[bass.ds(i, 1), :, :])
                    dst_t = sb.tile([128, K1], I32, tag="dstt", bufs=2)
                    nc.sync.dma_start(dst_t[:], cdst[bass.ds(i, 1), :, :])
                    ed_p = sb.tile([128, HEADS], BF16, tag="edp", bufs=2)
                    nc.sync.dma_start(ed_p[:], tin[bass.ds(i, 1), :, OUT1 + HEADS:TW])
                    sel = sb.tile([128, K1, 128], BF16, tag="sel", bufs=2)
                    nc.vector.tensor_tensor(
                        sel[:], dst_t[:, :, None].to_broadcast([128, K1, 128]),
                        iota_t[:, None, :].to_broadcast([128, K1, 128]),
                        mybir.AluOpType.is_equal)
                    nump = ps.tile([128, OUT1], F32, tag="nump", bufs=1)
                    denp = ps.tile([128, HEADS], F32, tag="denp", bufs=1)
                    for k in range(K1):
                        g = sb.tile([128, TW], BF16, tag="g", bufs=4)
                        nc.gpsimd.indirect_dma_start(
                            out=g[:], out_offset=None, in_=T[:, :],
                            in_offset=bass.IndirectOffsetOnAxis(
                                ap=src_t[:, k:k + 1], axis=0))
                        stp = ps.tile([128, 128], BF16, tag="stp", bufs=2)
                        nc.tensor.transpose(stp[:], sel[:, k, :], ident_t[:])
                        selT = sb.tile([128, 128], BF16, tag="selT", bufs=2)
                        nc.vector.tensor_copy(selT[:], stp[:])
                        edst = ps.tile([128, HEADS], F32, tag="edst", bufs=2)
                        nc.tensor.matmul(edst[:], lhsT=selT[:], rhs=ed_p[:],
                                         start=True, stop=True)
                        z = sb.tile([128, HEADS], F32, tag="z", bufs=2)
                        nc.vector.tensor_tensor(
                            z[:], g[:, OUT1:OUT1 + HEADS], edst[:],
                            mybir.AluOpType.add)
                        z2 = sb.tile([128, HEADS], F32, tag="z2", bufs=2)
                        nc.vector.tensor_scalar_mul(z2[:], z[:], NEG)
                        nc.vector.tensor_tensor(z[:], z[:], z2[:],
                                                mybir.AluOpType.max)
                        a = sb.tile([128, HEADS], F32, tag="a", bufs=2)
                        nc.scalar.activation(a[:], z[:],
                                             mybir.ActivationFunctionType.Exp)
                        abf = sb.tile([128, HEADS], BF16, tag="abf", bufs=2)
                        nc.vector.tensor_copy(abf[:], a[:])
                        msg = sb.tile([128, OUT1], BF16, tag="msg", bufs=2)
                        for h in range(HEADS):
                            nc.vector.tensor_scalar_mul(
                                msg[:, h * HID:(h + 1) * HID],
                                g[:, h * HID:(h + 1) * HID], a[:, h:h + 1])
                        nc.tensor.matmul(nump[:], lhsT=sel[:, k, :], rhs=msg[:],
                                         start=(k == 0), stop=(k == K1 - 1))
                        nc.tensor.matmul(denp[:], lhsT=sel[:, k, :], rhs=abf[:],
                                         start=(k == 0), stop=(k == K1 - 1))
                    dcl = sb.tile([128, HEADS], F32, tag="dcl", bufs=2)
                    nc.vector.tensor_scalar_max(dcl[:], denp[:], 1e-6)
                    dr = sb.tile([128, HEADS], F32, tag="dr", bufs=2)
                    nc.vector.reciprocal(dr[:], dcl[:])
                    outc = sb.tile([128, OUT1], BF16, tag="outc", bufs=2)
                    for h in range(HEADS):
                        nc.vector.tensor_scalar_mul(
                            outc[:, h * HID:(h + 1) * HID],
                            nump[:, h * HID:(h + 1) * HID], dr[:, h:h + 1])
                    nc.vector.tensor_scalar_max(outc[:], outc[:], 0.0)
                    rt = sb.tile([128, OUT1], BF16, tag="rt", bufs=2)
                    nc.vector.tensor_scalar_mul(rt[:], outc[:], 0.5)
                    nc.sync.dma_start(yin[bass.ds(i, 1), :, :], outc[:])
                    nc.sync.dma_start(res[bass.ds(i, 1), :, :], rt[:])

        def lp_round(Y, res, yout):
            """yout = clip(sum_e w*Y[src] + res, 0, 1), panels of 128 dst."""
            with ExitStack() as c2:
                pn = phase_n[0]; phase_n[0] += 1
                sb = c2.enter_context(tc.tile_pool(name=f"lpsb{pn}", bufs=2))
                ps = c2.enter_context(tc.tile_pool(name=f"lpps{pn}", bufs=1, space="PSUM"))
                with tc.For_i(0, PPC, 1) as i:
                    src_t = sb.tile([128, K2], I32, tag="lsrct", bufs=2)
                    nc.sync.dma_start(src_t[:], lsrc[bass.ds(i, 1), :, :])
                    dst_t = sb.tile([128, K2], I32, tag="ldstt", bufs=2)
                    nc.sync.dma_start(dst_t[:], ldst[bass.ds(i, 1), :, :])
                    w_t = sb.tile([128, K2], F32, tag="lwt", bufs=2)
                    nc.sync.dma_start(w_t[:], lw[bass.ds(i, 1), :, :])
                    res_t = sb.tile([128, OUT1], BF16, tag="lrest", bufs=2)
                    nc.sync.dma_start(res_t[:], res[bass.ds(i, 1), :, :])
                    sel = sb.tile([128, K2, 128], BF16, tag="lsel", bufs=2)
                    nc.vector.tensor_tensor(
                        sel[:], dst_t[:, :, None].to_broadcast([128, K2, 128]),
                        iota_t[:, None, :].to_broadcast([128, K2, 128]),
                        mybir.AluOpType.is_equal)
                    aggp = ps.tile([128, OUT1], F32, tag="aggp", bufs=1)
                    for k in range(K2):
                        g = sb.tile([128, OUT1], BF16, tag="lg", bufs=4)
                        nc.gpsimd.indirect_dma_start(
                            out=g[:], out_offset=None, in_=Y[:, :],
                            in_offset=bass.IndirectOffsetOnAxis(
                                ap=src_t[:, k:k + 1], axis=0))
                        msg = sb.tile([128, OUT1], BF16, tag="lmsg", bufs=2)
                        nc.vector.tensor_scalar_mul(msg[:], g[:], w_t[:, k:k + 1])
                        nc.tensor.matmul(aggp[:], lhsT=sel[:, k, :], rhs=msg[:],
                                         start=(k == 0), stop=(k == K2 - 1))
                    y_t = sb.tile([128, OUT1], BF16, tag="lyt", bufs=2)
                    nc.vector.tensor_tensor(y_t[:], aggp[:], res_t[:],
                                            mybir.AluOpType.add)
                    from concourse import mybir as _mb
                    nc.vector.tensor_scalar(y_t[:], y_t[:], 1.0, 0.0,
                                            op0=_mb.AluOpType.min,
                                            op1=_mb.AluOpType.max)
                    nc.sync.dma_start(yout[bass.ds(i, 1), :, :], y_t[:])

        # ---- phase 1: T1 = x @ [W1|wes1|wed1] (shard) + AG ----
        def load_x_lhs(lt, j, q):
            nc.sync.dma_start(lt[:], xTs[q, :, j, :])
        gemm(t1in, load_x_lhs, wcat1, 2)
        allgather(t1in, T1)

        # ---- phase 2: conv1 aggregation + AG ----
        conv_agg(T1, t1in, y0in, res1)
        allgather(y0in, Y0)

        # ---- phase 3/4: LP rounds for conv1 ----
        lp_round(Y0, res1, y1ain)
        allgather(y1ain, Y1a)
        lp_round(Y1a, res1, y1bin)

        # ---- phase 5: T2 = h1 @ [W2|wes2|wed2] (shard, transpose lhs) + AG ----
        def load_h_lhs(lt, j, q):
            nc.sync.dma_start(lt[:], y1bin[j, :, q * 128:(q + 1) * 128],
                              transpose=True)
        gemm(t2in, load_h_lhs, wcat2, 4)
        allgather(t2in, T2)

        # ---- phase 6: conv2 aggregation + AG ----
        conv_agg(T2, t2in, y20in, res2)
        allgather(y20in, Y20)

        # ---- phase 7/8: LP rounds for conv2 ----
        lp_round(Y20, res2, y2ain)
        allgather(y2ain, Y2a)
        lp_round(Y2a, res2, y2bin)

        # ---- phase 9: pooling (partial sums over this core's nodes) ----
        with ExitStack() as c2:
            sb = c2.enter_context(tc.tile_pool(name="posb", bufs=3))
            ps = c2.enter_context(tc.tile_pool(name="pops", bufs=1, space="PSUM"))
            psA = ps.tile([64, IN_CH], F32, tag="psA", bufs=1)
            psB = ps.tile([64, OUT1], F32, tag="psB", bufs=1)
            psC = ps.tile([64, OUT1], F32, tag="psC", bufs=1)
            for j in range(PPC):
                b_t = sb.tile([128, 1], I32, tag="bt", bufs=2)
                nc.sync.dma_start(b_t[:], batchl[j])
                selp = sb.tile([128, 64], BF16, tag="selp", bufs=2)
                nc.vector.tensor_tensor(
                    selp[:], b_t[:, 0:1].to_broadcast([128, 64]),
                    iota_t[:, 0:64], mybir.AluOpType.is_equal)
                x_t = sb.tile([128, IN_CH], BF16, tag="xt", bufs=2)
                nc.sync.dma_start(x_t[:], xs[j])
                h1_t = sb.tile([128, OUT1], BF16, tag="h1t", bufs=2)
                nc.sync.dma_start(h1_t[:], y1bin[j])
                h2_t = sb.tile([128, OUT1], BF16, tag="h2t", bufs=2)
                nc.sync.dma_start(h2_t[:], y2bin[j])
                nc.tensor.matmul(psA[:], lhsT=selp[:], rhs=x_t[:],
                                 start=(j == 0), stop=(j == PPC - 1))
                nc.tensor.matmul(psB[:], lhsT=selp[:], rhs=h1_t[:],
                                 start=(j == 0), stop=(j == PPC - 1))
                nc.tensor.matmul(psC[:], lhsT=selp[:], rhs=h2_t[:],
                                 start=(j == 0), stop=(j == PPC - 1))
            oA = sb.tile([64, IN_CH], F32, tag="oA")
            nc.vector.tensor_copy(oA[:], psA[:])
            nc.sync.dma_start(out_pool[:, 0:IN_CH], oA[:])
            oB = sb.tile([64, OUT1], F32, tag="oB")
            nc.vector.tensor_copy(oB[:], psB[:])
            nc.sync.dma_start(out_pool[:, IN_CH:IN_CH + OUT1], oB[:])
            oC = sb.tile([64, OUT1], F32, tag="oC")
            nc.vector.tensor_copy(oC[:], psC[:])
            nc.sync.dma_start(out_pool[:, IN_CH + OUT1:1280], oC[:])

    nc.compile()
    return nc


def _build_edge_panels(src, dst, weights=None):
    """Sort edges by dst, pack into per-panel [128, K] tiles (padded)."""
    order = np.argsort(dst, kind="stable")
    s = src[order].astype(np.int64)
    d = dst[order].astype(np.int64)
    w = weights[order].astype(np.float32) if weights is not None else None
    P = N_PAD // 128
    starts = np.searchsorted(d, np.arange(0, N_PAD + 1, 128))
    counts = np.diff(starts)
    K = max(1, int(np.ceil(counts.max() / 128)))
    S = np.zeros((P, K * 128), np.int32)
    D = np.full((P, K * 128), 200, np.int32)
    W = np.zeros((P, K * 128), np.float32) if w is not None else None
    pid = d // 128
    pos = np.arange(len(d)) - starts[pid]
    S[pid, pos] = s
    D[pid, pos] = d % 128
    if w is not None:
        W[pid, pos] = w

    def lay(A):
        return np.ascontiguousarray(
            A.reshape(P, K, 128).transpose(0, 2, 1))

    return lay(S), lay(D), (lay(W) if w is not None else None), K


def kernel(x, edge_index, batch,
           conv1_W, conv1_asrc, conv1_adst, conv1_b,
           conv2_W, conv2_asrc, conv2_adst, conv2_b,
           mlp_W1, mlp_b1, mlp_W2, mlp_b2):
    from concourse.bass_utils import run_bass_kernel_spmd
    bf16 = ml_dtypes.bfloat16

    x = np.asarray(x, np.float32)
    edge_index = np.asarray(edge_index)
    batch = np.asarray(batch).astype(np.int64)
    n = x.shape[0]
    src = edge_index[0].astype(np.int64)
    dst = edge_index[1].astype(np.int64)

    # conv edges = raw + self loops; LP edges = raw with sym-norm weights
    loop = np.arange(n, dtype=np.int64)
    c_src = np.concatenate([src, loop])
    c_dst = np.concatenate([dst, loop])
    deg = np.bincount(dst, minlength=n).astype(np.float32)
    dis = np.where(deg > 0, 1.0 / np.sqrt(np.maximum(deg, 1.0)), 0.0)
    wlp = LP_ALPHA * dis[src] * dis[dst]

    cS, cD, _, K1 = _build_edge_panels(c_src, c_dst)
    lS, lD, lW, K2 = _build_edge_panels(src, dst, wlp)

    # folded attention-logit weights
    def fold(W, a):
        a = np.asarray(a, np.float32)
        return np.stack([W[:, h * HID:(h + 1) * HID] @ a[h]
                         for h in range(HEADS)], axis=1)

    conv1_W = np.asarray(conv1_W, np.float32)
    conv2_W = np.asarray(conv2_W, np.float32)
    wc1 = np.concatenate([conv1_W, fold(conv1_W, conv1_asrc),
                          fold(conv1_W, conv1_adst)], axis=1)   # [256, 520]
    wc2 = np.concatenate([conv2_W, fold(conv2_W, conv2_asrc),
                          fold(conv2_W, conv2_adst)], axis=1)   # [512, 520]
    wc1_dev = np.ascontiguousarray(
        wc1.reshape(2, 128, TW)).astype(bf16)
    wc2_dev = np.ascontiguousarray(
        wc2.reshape(4, 128, TW)).astype(bf16)

    xp = np.zeros((N_PAD, IN_CH), np.float32)
    xp[:n] = x
    xb = xp.astype(bf16)
    xT = np.ascontiguousarray(xp.T).astype(bf16)      # [256, N_PAD]

    batch_p = np.full(N_PAD, 200, np.int64)
    batch_p[:n] = batch

    iota = np.tile(np.arange(128, dtype=np.int32), (128, 1))
    ident = np.eye(128, dtype=bf16)

    key = (K1, K2)
    if _cached.get("key") != key:
        _cached["nc"] = _build_program(K1, K2)
        _cached["key"] = key
    nc = _cached["nc"]

    in_maps = []
    for c in range(N_CORES):
        r0, r1 = c * NC_PAD, (c + 1) * NC_PAD
        p0, p1 = c * PPC, (c + 1) * PPC
        in_maps.append({
            "xTs": np.ascontiguousarray(
                xT[:, r0:r1].reshape(2, 128, PPC, 128)),
            "xs": np.ascontiguousarray(xb[r0:r1].reshape(PPC, 128, IN_CH)),
            "wcat1": wc1_dev, "wcat2": wc2_dev,
            "csrc": cS[p0:p1], "cdst": cD[p0:p1],
            "lsrc": lS[p0:p1], "ldst": lD[p0:p1],
            "lw": lW[p0:p1],
            "batchl": batch_p[r0:r1].reshape(PPC, 128, 1).astype(np.int32),
            "iota": iota, "ident": ident,
        })

    import time
    t0 = time.time()
    res = run_bass_kernel_spmd(nc, in_maps, core_ids=list(range(N_CORES)))
    _cached["device_wall_ns"] = int((time.time() - t0) * 1e9)
    _cached["last_result"] = res

    pooled_sum = np.zeros((64, 1280), np.float64)
    for c in range(N_CORES):
        pooled_sum += res.results[c]["out_pool"].astype(np.float64)
    cnts = np.bincount(batch, minlength=N_GRAPHS).astype(np.float32)
    pooled = (pooled_sum.astype(np.float32)
              / np.maximum(cnts, 1.0)[:, None])

    hdd = np.maximum(pooled @ np.asarray(mlp_W1, np.float32)
                     + np.asarray(mlp_b1, np.float32), 0.0)
    out = hdd @ np.asarray(mlp_W2, np.float32) + np.asarray(mlp_b2, np.float32)
    # conv biases: zero in this model; fold nonzero biases on host if present
    b1 = np.asarray(conv1_b, np.float32)
    b2 = np.asarray(conv2_b, np.float32)
    if np.any(b1) or np.any(b2):
        raise NotImplementedError("nonzero conv bias not folded")
    return out.astype(np.float32)


# revision 7
# speedup vs baseline: 61.7010x; 56.1762x over previous
"""DSGIAT GraphBranch kernel for trn2, 8 NeuronCores, full model on device.

Design: node-sharded (3840 padded nodes/core). Edge aggregation is done per
128-dst-node panel: gather source rows via indirect DMA, build a 0/1
selection matrix from dst indices (is_equal vs iota), and segment-sum via
TensorE matmul (sel.T @ msg). Stage boundaries that need the full node table
(conv outputs feeding LP gathers, GEMM tables feeding conv gathers) are
replicated via AllGather. Host does edge sorting/packing and the tiny MLP.
"""
import numpy as np
import ml_dtypes
from contextlib import ExitStack

N_NODES = 30000
N_PAD = 30720            # 8 * 3840
N_CORES = 8
NC_PAD = N_PAD // N_CORES     # 3840 rows per core
PPC = NC_PAD // 128           # 30 panels per core
IN_CH = 256
HID = 128
HEADS = 4
OUT1 = 512
TW = OUT1 + 2 * HEADS         # 520 table width: [h | es | ed]
N_GRAPHS = 64
LP_ALPHA = 0.5
NEG = 0.2

_cached = {}


def _build_program(K1, K2):
    import concourse.tile as tile
    from concourse import bacc, bass, mybir

    BF16 = mybir.dt.bfloat16
    F32 = mybir.dt.float32
    I32 = mybir.dt.int32

    nc = bacc.Bacc("TRN2", target_bir_lowering=False, debug=False,
                   num_devices=N_CORES)

    # ---- inputs (per core) ----
    xs = nc.dram_tensor("xs", [PPC, 128, IN_CH], BF16, kind="ExternalInput")
    wcat1 = nc.dram_tensor("wcat1", [2, 128, TW], BF16, kind="ExternalInput")
    wcat2 = nc.dram_tensor("wcat2", [4, 128, TW], BF16, kind="ExternalInput")
    csrc = nc.dram_tensor("csrc", [PPC, 128, K1], I32, kind="ExternalInput")
    cdst = nc.dram_tensor("cdst", [PPC, 128, K1], I32, kind="ExternalInput")
    lsrc = nc.dram_tensor("lsrc", [PPC, 128, K2], I32, kind="ExternalInput")
    ldst = nc.dram_tensor("ldst", [PPC, 128, K2], I32, kind="ExternalInput")
    lw = nc.dram_tensor("lw", [PPC, 128, K2], F32, kind="ExternalInput")
    batchl = nc.dram_tensor("batchl", [PPC, 128, 1], I32, kind="ExternalInput")
    iota = nc.dram_tensor("iota", [128, 128], I32, kind="ExternalInput")
    ident = nc.dram_tensor("ident", [128, 128], BF16, kind="ExternalInput")

    out_pool = nc.dram_tensor("out_pool", [64, 1280], F32, kind="ExternalOutput")

    RG = [list(range(N_CORES))]

    # persistent DRAM scratch (plain Internal tensors: indirect DMA needs
    # zero-offset APs)
    t1in = nc.dram_tensor("t1in", [PPC, 128, TW], BF16, kind="Internal")
    T1 = nc.dram_tensor("T1", [N_PAD, TW], BF16, kind="Internal", addr_space="Shared")
    y0in = nc.dram_tensor("y0in", [PPC, 128, OUT1], BF16, kind="Internal")
    Y0 = nc.dram_tensor("Y0", [N_PAD, OUT1], BF16, kind="Internal", addr_space="Shared")
    y1ain = nc.dram_tensor("y1ain", [PPC, 128, OUT1], BF16, kind="Internal")
    Y1a = nc.dram_tensor("Y1a", [N_PAD, OUT1], BF16, kind="Internal", addr_space="Shared")
    y1bin = nc.dram_tensor("y1bin", [PPC, 128, OUT1], BF16, kind="Internal")
    t2in = nc.dram_tensor("t2in", [PPC, 128, TW], BF16, kind="Internal")
    T2 = nc.dram_tensor("T2", [N_PAD, TW], BF16, kind="Internal", addr_space="Shared")
    y20in = nc.dram_tensor("y20in", [PPC, 128, OUT1], BF16, kind="Internal")
    Y20 = nc.dram_tensor("Y20", [N_PAD, OUT1], BF16, kind="Internal", addr_space="Shared")
    y2ain = nc.dram_tensor("y2ain", [PPC, 128, OUT1], BF16, kind="Internal")
    Y2a = nc.dram_tensor("Y2a", [N_PAD, OUT1], BF16, kind="Internal", addr_space="Shared")
    y2bin = nc.dram_tensor("y2bin", [PPC, 128, OUT1], BF16, kind="Internal")
    res1 = nc.dram_tensor("res1", [PPC, 128, OUT1], BF16, kind="Internal")
    res2 = nc.dram_tensor("res2", [PPC, 128, OUT1], BF16, kind="Internal")

    phase_n = [0]

    with tile.TileContext(nc) as tc, ExitStack() as ctx:
        cpool = ctx.enter_context(tc.tile_pool(name="cpool", bufs=1))

        # constants
        iota_t = cpool.tile([128, 128], I32)
        nc.sync.dma_start(iota_t[:], iota[:])
        ident_t = cpool.tile([128, 128], BF16)
        nc.sync.dma_start(ident_t[:], ident[:])

        def gemm(dst_dram, lhs_loader, w_dram, kt):
            """dst_dram[j] = lhsT_j.T @ wcat  for j in range(PPC)."""
            with ExitStack() as c2:
                pn = phase_n[0]; phase_n[0] += 1
                sb = c2.enter_context(tc.tile_pool(name=f"gsb{pn}", bufs=3))
                wp = c2.enter_context(tc.tile_pool(name=f"gwp{pn}", bufs=1))
                ps = c2.enter_context(tc.tile_pool(name=f"gps{pn}", bufs=1, space="PSUM"))
                w_t = wp.tile([128, kt, TW], BF16)
                for q in range(kt):
                    nc.sync.dma_start(w_t[:, q, :], w_dram[q])
                for j in range(PPC):
                    p1 = ps.tile([128, OUT1], F32, tag="p1", bufs=2)
                    p2 = ps.tile([128, 8], F32, tag="p2", bufs=2)
                    for q in range(kt):
                        lt = sb.tile([128, 128], BF16, tag="lt", bufs=3)
                        lhs_loader(lt, j, q)
                        nc.tensor.matmul(p1[:], lhsT=lt[:], rhs=w_t[:, q, 0:OUT1],
                                         start=(q == 0), stop=(q == kt - 1))
                        nc.tensor.matmul(p2[:], lhsT=lt[:], rhs=w_t[:, q, OUT1:TW],
                                         start=(q == 0), stop=(q == kt - 1))
                    st = sb.tile([128, TW], BF16, tag="st", bufs=3)
                    nc.scalar.copy(st[:, 0:OUT1], p1[:])
                    nc.scalar.copy(st[:, OUT1:TW], p2[:])
                    nc.sync.dma_start(dst_dram[j], st[:])

        def allgather(src3, dst2):
            nc.gpsimd.collective_compute(
                "AllGather", mybir.AluOpType.bypass, replica_groups=RG,
                ins=[src3[:].opt()], outs=[dst2[:].opt()])

        def conv_agg(T, tin, yin, res):
            """GAT aggregation: panels of 128 dst, K1 edge tiles each."""
            with ExitStack() as c2:
                pn = phase_n[0]; phase_n[0] += 1
                sb = c2.enter_context(tc.tile_pool(name=f"casb{pn}", bufs=2))
                ps = c2.enter_context(tc.tile_pool(name=f"caps{pn}", bufs=1, space="PSUM"))
                with tc.For_i(0, PPC, 1) as i:
                    src_t = sb.tile([128, K1], I32, tag="srct", bufs=2)
                    nc.sync.dma_start(src_t[:], csrc[bass.ds(i, 1), :, :])
                    dst_t = sb.tile([128, K1], I32, tag="dstt", bufs=2)
                    nc.sync.dma_start(dst_t[:], cdst[bass.ds(i, 1), :, :])
                    ed_p = sb.tile([128, HEADS], BF16, tag="edp", bufs=2)
                    nc.sync.dma_start(ed_p[:], tin[bass.ds(i, 1), :, OUT1 + HEADS:TW])
                    sel = sb.tile([128, K1, 128], BF16, tag="sel", bufs=2)
                    nc.vector.tensor_tensor(
                        sel[:], dst_t[:, :, None].to_broadcast([128, K1, 128]),
                        iota_t[:, None, :].to_broadcast([128, K1, 128]),
                        mybir.AluOpType.is_equal)
                    nump = ps.tile([128, OUT1], F32, tag="nump", bufs=1)
                    denp = ps.tile([128, HEADS], F32, tag="denp", bufs=1)
                    for k in range(K1):
                        g = sb.tile([128, TW], BF16, tag="g", bufs=4)
                        nc.gpsimd.indirect_dma_start(
                            out=g[:], out_offset=None, in_=T[:, :],
                            in_offset=bass.IndirectOffsetOnAxis(
                                ap=src_t[:, k:k + 1], axis=0))
                        stp = ps.tile([128, 128], BF16, tag="stp", bufs=2)
                        nc.tensor.transpose(stp[:], sel[:, k, :], ident_t[:])
                        selT = sb.tile([128, 128], BF16, tag="selT", bufs=2)
                        nc.vector.tensor_copy(selT[:], stp[:])
                        edst = ps.tile([128, HEADS], F32, tag="edst", bufs=2)
                        nc.tensor.matmul(edst[:], lhsT=selT[:], rhs=ed_p[:],
                                         start=True, stop=True)
                        z = sb.tile([128, HEADS], F32, tag="z", bufs=2)
                        nc.vector.tensor_tensor(
                            z[:], g[:, OUT1:OUT1 + HEADS], edst[:],
                            mybir.AluOpType.add)
                        z2 = sb.tile([128, HEADS], F32, tag="z2", bufs=2)
                        nc.vector.tensor_scalar_mul(z2[:], z[:], NEG)
                        nc.vector.tensor_tensor(z[:], z[:], z2[:],
                                                mybir.AluOpType.max)
                        a = sb.tile([128, HEADS], F32, tag="a", bufs=2)
                        nc.scalar.activation(a[:], z[:],
                                             mybir.ActivationFunctionType.Exp)
                        abf = sb.tile([128, HEADS], BF16, tag="abf", bufs=2)
                        nc.vector.tensor_copy(abf[:], a[:])
                        msg = sb.tile([128, OUT1], BF16, tag="msg", bufs=2)
                        for h in range(HEADS):
                            nc.vector.tensor_scalar_mul(
                                msg[:, h * HID:(h + 1) * HID],
                                g[:, h * HID:(h + 1) * HID], a[:, h:h + 1])
                        nc.tensor.matmul(nump[:], lhsT=sel[:, k, :], rhs=msg[:],
                                         start=(k == 0), stop=(k == K1 - 1))
                        nc.tensor.matmul(denp[:], lhsT=sel[:, k, :], rhs=abf[:],
                                         start=(k == 0), stop=(k == K1 - 1))
                    dcl = sb.tile([128, HEADS], F32, tag="dcl", bufs=2)
                    nc.vector.tensor_scalar_max(dcl[:], denp[:], 1e-6)
                    dr = sb.tile([128, HEADS], F32, tag="dr", bufs=2)
                    nc.vector.reciprocal(dr[:], dcl[:])
                    outc = sb.tile([128, OUT1], BF16, tag="outc", bufs=2)
                    for h in range(HEADS):
                        nc.vector.tensor_scalar_mul(
                            outc[:, h * HID:(h + 1) * HID],
                            nump[:, h * HID:(h + 1) * HID], dr[:, h:h + 1])
                    nc.vector.tensor_scalar_max(outc[:], outc[:], 0.0)
                    rt = sb.tile([128, OUT1], BF16, tag="rt", bufs=2)
                    nc.vector.tensor_scalar_mul(rt[:], outc[:], 0.5)
                    nc.sync.dma_start(yin[bass.ds(i, 1), :, :], outc[:])
                    nc.sync.dma_start(res[bass.ds(i, 1), :, :], rt[:])

        def lp_round(Y, res, yout):
            """yout = clip(sum_e w*Y[src] + res, 0, 1), panels of 128 dst."""
            with ExitStack() as c2:
                pn = phase_n[0]; phase_n[0] += 1
                sb = c2.enter_context(tc.tile_pool(name=f"lpsb{pn}", bufs=2))
                ps = c2.enter_context(tc.tile_pool(name=f"lpps{pn}", bufs=1, space="PSUM"))
                with tc.For_i(0, PPC, 1) as i:
                    src_t = sb.tile([128, K2], I32, tag="lsrct", bufs=2)
                    nc.sync.dma_start(src_t[:], lsrc[bass.ds(i, 1), :, :])
                    dst_t = sb.tile([128, K2], I32, tag="ldstt", bufs=2)
                    nc.sync.dma_start(dst_t[:], ldst[bass.ds(i, 1), :, :])
                    w_t = sb.tile([128, K2], F32, tag="lwt", bufs=2)
                    nc.sync.dma_start(w_t[:], lw[bass.ds(i, 1), :, :])
                    res_t = sb.tile([128, OUT1], BF16, tag="lrest", bufs=2)
                    nc.sync.dma_start(res_t[:], res[bass.ds(i, 1), :, :])
                    sel = sb.tile([128, K2, 128], BF16, tag="lsel", bufs=2)
                    nc.vector.tensor_tensor(
                        sel[:], dst_t[:, :, None].to_broadcast([128, K2, 128]),
                        iota_t[:, None, :].to_broadcast([128, K2, 128]),
                        mybir.AluOpType.is_equal)
                    aggp = ps.tile([128, OUT1], F32, tag="aggp", bufs=1)
                    for k in range(K2):
                        g = sb.tile([128, OUT1], BF16, tag="lg", bufs=4)
                        nc.gpsimd.indirect_dma_start(
                            out=g[:], out_offset=None, in_=Y[:, :],
                            in_offset=bass.IndirectOffsetOnAxis(
                                ap=src_t[:, k:k + 1], axis=0))
                        msg = sb.tile([128, OUT1], BF16, tag="lmsg", bufs=2)
                        nc.vector.tensor_scalar_mul(msg[:], g[:], w_t[:, k:k + 1])
                        nc.tensor.matmul(aggp[:], lhsT=sel[:, k, :], rhs=msg[:],
                                         start=(k == 0), stop=(k == K2 - 1))
                    y_t = sb.tile([128, OUT1], BF16, tag="lyt", bufs=2)
                    nc.vector.tensor_tensor(y_t[:], aggp[:], res_t[:],
                                            mybir.AluOpType.add)
                    from concourse import mybir as _mb
                    nc.vector.tensor_scalar(y_t[:], y_t[:], 1.0, 0.0,
                                            op0=_mb.AluOpType.min,
                                            op1=_mb.AluOpType.max)
                    nc.sync.dma_start(yout[bass.ds(i, 1), :, :], y_t[:])

        B = tc.strict_bb_all_engine_barrier

        # ---- phase 1: T1 = x @ [W1|wes1|wed1] (shard) + AG ----
        def load_x_lhs(lt, j, q):
            nc.sync.dma_start(lt[:], xs[j, :, q * 128:(q + 1) * 128],
                              transpose=True)
        gemm(t1in, load_x_lhs, wcat1, 2)
        B()
        allgather(t1in, T1)
        B()

        # ---- phase 2: conv1 aggregation + AG ----
        conv_agg(T1, t1in, y0in, res1)
        B()
        allgather(y0in, Y0)
        B()

        # ---- phase 3/4: LP rounds for conv1 ----
        lp_round(Y0, res1, y1ain)
        B()
        allgather(y1ain, Y1a)
        B()
        lp_round(Y1a, res1, y1bin)
        B()

        # ---- phase 5: T2 = h1 @ [W2|wes2|wed2] (shard, transpose lhs) + AG ----
        def load_h_lhs(lt, j, q):
            nc.sync.dma_start(lt[:], y1bin[j, :, q * 128:(q + 1) * 128],
                              transpose=True)
        gemm(t2in, load_h_lhs, wcat2, 4)
        B()
        allgather(t2in, T2)
        B()

        # ---- phase 6: conv2 aggregation + AG ----
        conv_agg(T2, t2in, y20in, res2)
        B()
        allgather(y20in, Y20)
        B()

        # ---- phase 7/8: LP rounds for conv2 ----
        lp_round(Y20, res2, y2ain)
        B()
        allgather(y2ain, Y2a)
        B()
        lp_round(Y2a, res2, y2bin)
        B()

        # ---- phase 9: pooling (partial sums over this core's nodes) ----
        with ExitStack() as c2:
            sb = c2.enter_context(tc.tile_pool(name="posb", bufs=3))
            ps = c2.enter_context(tc.tile_pool(name="pops", bufs=1, space="PSUM"))
            psA = ps.tile([64, IN_CH], F32, tag="psA", bufs=1)
            psB = ps.tile([64, OUT1], F32, tag="psB", bufs=1)
            psC = ps.tile([64, OUT1], F32, tag="psC", bufs=1)
            for j in range(PPC):
                b_t = sb.tile([128, 1], I32, tag="bt", bufs=2)
                nc.sync.dma_start(b_t[:], batchl[j])
                selp = sb.tile([128, 64], BF16, tag="selp", bufs=2)
                nc.vector.tensor_tensor(
                    selp[:], b_t[:, 0:1].to_broadcast([128, 64]),
                    iota_t[:, 0:64], mybir.AluOpType.is_equal)
                x_t = sb.tile([128, IN_CH], BF16, tag="xt", bufs=2)
                nc.sync.dma_start(x_t[:], xs[j])
                h1_t = sb.tile([128, OUT1], BF16, tag="h1t", bufs=2)
                nc.sync.dma_start(h1_t[:], y1bin[j])
                h2_t = sb.tile([128, OUT1], BF16, tag="h2t", bufs=2)
                nc.sync.dma_start(h2_t[:], y2bin[j])
                nc.tensor.matmul(psA[:], lhsT=selp[:], rhs=x_t[:],
                                 start=(j == 0), stop=(j == PPC - 1))
                nc.tensor.matmul(psB[:], lhsT=selp[:], rhs=h1_t[:],
                                 start=(j == 0), stop=(j == PPC - 1))
                nc.tensor.matmul(psC[:], lhsT=selp[:], rhs=h2_t[:],
                                 start=(j == 0), stop=(j == PPC - 1))
            oA = sb.tile([64, IN_CH], F32, tag="oA")
            nc.vector.tensor_copy(oA[:], psA[:])
            nc.sync.dma_start(out_pool[:, 0:IN_CH], oA[:])
            oB = sb.tile([64, OUT1], F32, tag="oB")
            nc.vector.tensor_copy(oB[:], psB[:])
            nc.sync.dma_start(out_pool[:, IN_CH:IN_CH + OUT1], oB[:])
            oC = sb.tile([64, OUT1], F32, tag="oC")
            nc.vector.tensor_copy(oC[:], psC[:])
            nc.sync.dma_start(out_pool[:, IN_CH + OUT1:1280], oC[:])

    nc.compile()
    return nc


def _build_edge_panels(src, dst, weights=None):
    """Sort edges by dst, pack into per-panel [128, K] tiles (padded)."""
    order = np.argsort(dst, kind="stable")
    s = src[order].astype(np.int64)
    d = dst[order].astype(np.int64)
    w = weights[order].astype(np.float32) if weights is not None else None
    P = N_PAD // 128
    starts = np.searchsorted(d, np.arange(0, N_PAD + 1, 128))
    counts = np.diff(starts)
    K = max(1, int(np.ceil(counts.max() / 128)))
    S = np.zeros((P, K * 128), np.int32)
    D = np.full((P, K * 128), 200, np.int32)
    W = np.zeros((P, K * 128), np.float32) if w is not None else None
    pid = d // 128
    pos = np.arange(len(d)) - starts[pid]
    S[pid, pos] = s
    D[pid, pos] = d % 128
    if w is not None:
        W[pid, pos] = w

    def lay(A):
        return np.ascontiguousarray(
            A.reshape(P, K, 128).transpose(0, 2, 1))

    return lay(S), lay(D), (lay(W) if w is not None else None), K


def kernel(x, edge_index, batch,
           conv1_W, conv1_asrc, conv1_adst, conv1_b,
           conv2_W, conv2_asrc, conv2_adst, conv2_b,
           mlp_W1, mlp_b1, mlp_W2, mlp_b2):
    from concourse.bass_utils import run_bass_kernel_spmd
    bf16 = ml_dtypes.bfloat16

    x = np.asarray(x, np.float32)
    edge_index = np.asarray(edge_index)
    batch = np.asarray(batch).astype(np.int64)
    n = x.shape[0]
    src = edge_index[0].astype(np.int64)
    dst = edge_index[1].astype(np.int64)

    # conv edges = raw + self loops; LP edges = raw with sym-norm weights
    loop = np.arange(n, dtype=np.int64)
    c_src = np.concatenate([src, loop])
    c_dst = np.concatenate([dst, loop])
    deg = np.bincount(dst, minlength=n).astype(np.float32)
    dis = np.where(deg > 0, 1.0 / np.sqrt(np.maximum(deg, 1.0)), 0.0)
    wlp = LP_ALPHA * dis[src] * dis[dst]

    cS, cD, _, K1 = _build_edge_panels(c_src, c_dst)
    lS, lD, lW, K2 = _build_edge_panels(src, dst, wlp)

    # folded attention-logit weights
    def fold(W, a):
        a = np.asarray(a, np.float32)
        return np.stack([W[:, h * HID:(h + 1) * HID] @ a[h]
                         for h in range(HEADS)], axis=1)

    conv1_W = np.asarray(conv1_W, np.float32)
    conv2_W = np.asarray(conv2_W, np.float32)
    wc1 = np.concatenate([conv1_W, fold(conv1_W, conv1_asrc),
                          fold(conv1_W, conv1_adst)], axis=1)   # [256, 520]
    wc2 = np.concatenate([conv2_W, fold(conv2_W, conv2_asrc),
                          fold(conv2_W, conv2_adst)], axis=1)   # [512, 520]
    wc1_dev = np.ascontiguousarray(
        wc1.reshape(2, 128, TW)).astype(bf16)
    wc2_dev = np.ascontiguousarray(
        wc2.reshape(4, 128, TW)).astype(bf16)

    xp = np.zeros((N_PAD, IN_CH), np.float32)
    xp[:n] = x
    xb = xp.astype(bf16)

    batch_p = np.full(N_PAD, 200, np.int64)
    batch_p[:n] = batch

    iota = np.tile(np.arange(128, dtype=np.int32), (128, 1))
    ident = np.eye(128, dtype=bf16)

    key = (K1, K2)
    if _cached.get("key") != key:
        _cached["nc"] = _build_program(K1, K2)
        _cached["key"] = key
    nc = _cached["nc"]

    in_maps = []
    for c in range(N_CORES):
        r0, r1 = c * NC_PAD, (c + 1) * NC_PAD
        p0, p1 = c * PPC, (c + 1) * PPC
        in_maps.append({
            "xs": np.ascontiguousarray(xb[r0:r1].reshape(PPC, 128, IN_CH)),
            "wcat1": wc1_dev, "wcat2": wc2_dev,
            "csrc": cS[p0:p1], "cdst": cD[p0:p1],
            "lsrc": lS[p0:p1], "ldst": lD[p0:p1],
            "lw": lW[p0:p1],
            "batchl": batch_p[r0:r1].reshape(PPC, 128, 1).astype(np.int32),
            "iota": iota, "ident": ident,
        })

    import time
    t0 = time.time()
    res = run_bass_kernel_spmd(nc, in_maps, core_ids=list(range(N_CORES)))
    _cached["device_wall_ns"] = int((time.time() - t0) * 1e9)
    _cached["last_result"] = res

    pooled_sum = np.zeros((64, 1280), np.float64)
    for c in range(N_CORES):
        pooled_sum += res.results[c]["out_pool"].astype(np.float64)
    cnts = np.bincount(batch, minlength=N_GRAPHS).astype(np.float32)
    pooled = (pooled_sum.astype(np.float32)
              / np.maximum(cnts, 1.0)[:, None])

    hdd = np.maximum(pooled @ np.asarray(mlp_W1, np.float32)
                     + np.asarray(mlp_b1, np.float32), 0.0)
    out = hdd @ np.asarray(mlp_W2, np.float32) + np.asarray(mlp_b2, np.float32)
    # conv biases: zero in this model; fold nonzero biases on host if present
    b1 = np.asarray(conv1_b, np.float32)
    b2 = np.asarray(conv2_b, np.float32)
    if np.any(b1) or np.any(b2):
        raise NotImplementedError("nonzero conv bias not folded")
    return out.astype(np.float32)
